# revision 1
# baseline (speedup 1.0000x reference)
"""Trainium2 Bass kernel for GQA attention with RoPE (B=2, S=1024, HID=2048,
16 q heads / 4 kv heads, head dim 128, causal).

Sharding: 8 cores = 2 batches x 4 kv-head groups. Core c = b*4 + g handles
batch b and kv head g (query heads 4g..4g+3). Each core computes a partial
output y_part = attn_heads @ wo_shard; the host sums the 4 partials per batch.

v3: all-f16 dataflow (host casts inputs), PE transposes for x (f16 1c/r,
2x-mode copies), 2-deep phase-A pipeline, 2-heads-per-matmul
scores/den/PV (GQA heads share kv), diagonal-block causal skip, post-exp
multiplicative mask, fast reciprocal.
"""

import sys

import numpy as np

for _p in ("/opt/trn_rl_repo", "/root/.axon_site/_ro/trn_rl_repo"):
    if _p not in sys.path:
        sys.path.append(_p)

from contextlib import ExitStack

import concourse.bass as bass
import concourse.mybir as mybir
from concourse import bacc
from concourse.masks import make_identity
from concourse.tile import TileContext

P = 128           # partitions / head dim / seq chunk
S = 1024          # sequence length
HID = 2048        # model dim
NH = 4            # query heads per core
D = 128           # head dim
TQ = 256          # query macro-tile
NT = S // TQ      # 4 macro tiles
KC = HID // P     # 16 contraction chunks
NSK = S // P      # 8 key chunks
NG = S // P       # 8 row chunks
F16 = mybir.dt.float16
F32 = mybir.dt.float32
SCALE = 1.0 / float(np.sqrt(D))
AL = mybir.AluOpType
AF = mybir.ActivationFunctionType

N_CORES = 8
B = 2
N_KV = 4
H2 = D // 2


def build_nc(dbg=False):
    nc = bacc.Bacc("TRN2", target_bir_lowering=False, debug=False)
    x_d = nc.declare_dram_parameter("x", [S, HID], F16, isOutput=False)
    if dbg:
        ut_o = nc.declare_dram_parameter("ut_o", [P, NT * NH * TQ], F16, isOutput=True)
        rec_o = nc.declare_dram_parameter("rec_o", [P, NT * 2 * 2 * TQ], F32, isOutput=True)
        den_o = nc.declare_dram_parameter("den_o", [P, NT * 2 * 2 * TQ], F32, isOutput=True)
    cos_d = nc.declare_dram_parameter("cos", [S, D], F16, isOutput=False)
    sin_d = nc.declare_dram_parameter("sin", [S, D], F16, isOutput=False)
    wq_d = nc.declare_dram_parameter("wq", [HID, NH * D], F16, isOutput=False)
    wk_d = nc.declare_dram_parameter("wk", [HID, D], F16, isOutput=False)
    wv_d = nc.declare_dram_parameter("wv", [HID, D], F16, isOutput=False)
    wo_d = nc.declare_dram_parameter("wo", [NH * D, HID], F16, isOutput=False)
    out_d = nc.declare_dram_parameter("out", [S, HID], F16, isOutput=True)

    with TileContext(nc) as tc, ExitStack() as ctx:
        consts = ctx.enter_context(tc.tile_pool(name="consts", bufs=1))
        wpool = ctx.enter_context(tc.tile_pool(name="wpool", bufs=1))
        persist = ctx.enter_context(tc.tile_pool(name="persist", bufs=1))

        # ---- constants ----
        ident_f32 = consts.tile([P, P], F32, tag="ident_f32")
        make_identity(nc, ident_f32)
        ident = consts.tile([P, P], F16, tag="ident")
        nc.vector.tensor_copy(ident, ident_f32)
        ones = consts.tile([P, P], F16, tag="ones")
        nc.vector.memset(ones, 1.0)
        # causal 0/1 triangle: tri01[k, h, q] = (q >= k), f16, shared by both
        # diagonal chunks of every macro tile
        tri01 = consts.tile([P, 2, P], F16, tag="tri01")
        nc.gpsimd.memset(tri01, 1.0)
        nc.gpsimd.affine_select(
            out=tri01, in_=tri01, compare_op=AL.is_ge, fill=0.0,
            base=0, pattern=[[0, 2], [1, P]], channel_multiplier=-1,
        )

        # ---- persistent weights / tables / activations ----
        wq_sb = wpool.tile([P, KC, NH * D], F16, tag="wq")
        wq_r = wq_d[:].rearrange("(c p) n -> p c n", p=P)
        wkv_sb = wpool.tile([P, KC, 2 * D], F16, tag="wkv")
        wo_sb = wpool.tile([P, NH, HID], F16, tag="wo")
        wo_r = wo_d[:].rearrange("(h p) n -> p h n", p=P)
        cos_sb = wpool.tile([P, NG, D], F16, tag="cos")
        sin_sb = wpool.tile([P, NG, D], F16, tag="sin")

        qT_all = persist.tile([P, NH, S], F16, tag="qT")    # [d, h, sq]
        kT = persist.tile([P, S], F16, tag="kT")            # [d, sk]
        vv = persist.tile([P, NSK, D], F16, tag="vv")       # v natural [sk, d]

        # ---- pools ----
        pa = ctx.enter_context(tc.tile_pool(name="pa", bufs=2))
        pb = ctx.enter_context(tc.tile_pool(name="pb", bufs=2))
        ps_mega = ctx.enter_context(tc.tile_pool(name="ps_mega", bufs=7, space="PSUM"))
        ps_qkv = ctx.enter_context(tc.tile_pool(name="ps_qkv", bufs=1, space="PSUM"))

        # warm the PE clock gate while initial DMAs land
        warm_ps = ps_mega.tile([P, 512], F32, tag="mega", name="warm")
        for _ in range(24):
            nc.tensor.matmul(warm_ps[:, 0:P], ones, ones, start=True, stop=True)
        warm_drain = pa.tile([P, 4], F32, tag="warmdrain", bufs=1)
        nc.vector.tensor_copy(warm_drain, warm_ps[:, 0:4])

        # ---- DMAs: x natural per-chunk (sync queue), weights on the
        # scalar queue; x chunks transposed on the PE per phase-A stage. ----
        x_tiles = [None] * NG

        def emit_xdma(g):
            x_nat = pa.tile([P, HID], F16, tag="xnat", bufs=4)
            nc.sync.dma_start(out=x_nat, in_=x_d[g * P : (g + 1) * P, :])
            x_tiles[g] = x_nat

        # scalar queue: only cos/sin (small, needed early) - keeps the ACT
        # engine free for qkv/exp/y work. Everything else on the sync queue.
        emit_xdma(0)
        nc.scalar.dma_start(
            out=cos_sb, in_=cos_d[:].rearrange("(c p) d -> p c d", p=P)
        )
        nc.scalar.dma_start(
            out=sin_sb, in_=sin_d[:].rearrange("(c p) d -> p c d", p=P)
        )
        emit_xdma(1)
        nc.sync.dma_start(out=wq_sb[:, 0:8, :], in_=wq_r[:, 0:8, :])
        nc.sync.dma_start(out=wq_sb[:, 8:16, :], in_=wq_r[:, 8:16, :])
        emit_xdma(2)
        nc.sync.dma_start(
            out=wkv_sb[:, :, 0:D], in_=wk_d[:].rearrange("(c p) n -> p c n", p=P)
        )
        nc.sync.dma_start(
            out=wkv_sb[:, :, D : 2 * D],
            in_=wv_d[:].rearrange("(c p) n -> p c n", p=P),
        )
        wo_next = [0]

        def emit_wo_dma():
            h = wo_next[0]
            if h < NH:
                nc.sync.dma_start(out=wo_sb[:, h, :], in_=wo_r[:, h, :])
                wo_next[0] += 1

        def bcast_h(ap2d, n):
            """[P, w] slice -> [P, n, w] broadcast AP (0-stride head dim)."""
            return ap2d.rearrange("p (o w) -> p o w", o=1).to_broadcast(
                [P, n, ap2d.shape[-1]]
            )

        # ================= phase A stages =================
        def transposes(g):
            """x chunk -> xT chunk (PE transpose, f16; 2x-mode copy out)."""
            x_nat = x_tiles[g]
            xTg = pa.tile([P, KC, P], F16, tag="xT", bufs=3)
            xTg_flat = xTg.rearrange("p c d -> p (c d)")
            for kb in range(KC // 4):
                tp_ps = ps_mega.tile([P, 4 * P], F16, tag="mega", name="tp")
                for j in range(4):
                    k = 4 * kb + j
                    nc.tensor.transpose(
                        tp_ps[:, j * P : (j + 1) * P],
                        x_nat[:, k * P : (k + 1) * P],
                        ident,
                    )
                if kb % 2 == 0:
                    nc.vector.tensor_copy(
                        xTg_flat[:, kb * 4 * P : (kb + 1) * 4 * P], tp_ps
                    )
                else:
                    nc.scalar.activation(
                        out=xTg_flat[:, kb * 4 * P : (kb + 1) * 4 * P],
                        in_=tp_ps, func=AF.Copy,
                    )
            return xTg

        def proj(g, xTg):
            """q and kv projections for chunk g (PE, accumulating).
            q uses the dedicated 1-bank pool; kv borrows a mega slot so the
            attention phase gets a 7-deep mega rotation."""
            q_ps = ps_qkv.tile([P, NH * D], F32, tag="qkv")
            kv_ps = ps_mega.tile([P, 512], F32, tag="mega", name="kv")[:, 0 : 2 * D]
            for c in range(KC):
                nc.tensor.matmul(
                    q_ps, xTg[:, c, :], wq_sb[:, c, :],
                    start=(c == 0), stop=(c == KC - 1),
                )
            for c in range(KC):
                nc.tensor.matmul(
                    kv_ps, xTg[:, c, :], wkv_sb[:, c, :],
                    start=(c == 0), stop=(c == KC - 1),
                )
            qkv_sb = pa.tile([P, NH * D + 2 * D], F16, tag="qkvsb")
            nc.scalar.activation(out=qkv_sb[:, 0 : NH * D], in_=q_ps, func=AF.Copy)
            nc.scalar.activation(
                out=qkv_sb[:, NH * D : NH * D + 2 * D], in_=kv_ps, func=AF.Copy
            )
            return qkv_sb

        def rope_stage(g, qkv_sb):
            """RoPE on q heads (one 4-head strided pass) + k; v copy-out."""
            q3 = qkv_sb[:, 0 : NH * D].rearrange("p (h d) -> p h d", h=NH)
            k2 = qkv_sb[:, NH * D : NH * D + D]
            cos_g = cos_sb[:, g, :]
            sin_g = sin_sb[:, g, :]

            q_rope = pa.tile([P, NH, D], F16, tag="qrope")
            tmpq = pa.tile([P, NH, D], F16, tag="tmpq")
            nc.vector.scalar_tensor_tensor(
                out=tmpq[:, :, 0:H2], in0=q3[:, :, H2:D], scalar=-1.0,
                in1=bcast_h(sin_g[:, 0:H2], NH), op0=AL.mult, op1=AL.mult,
            )
            nc.vector.tensor_tensor(
                out=tmpq[:, :, H2:D], in0=q3[:, :, 0:H2],
                in1=bcast_h(sin_g[:, H2:D], NH), op=AL.mult,
            )
            nc.vector.tensor_tensor(
                out=q_rope, in0=q3, in1=bcast_h(cos_g, NH), op=AL.mult
            )
            nc.vector.tensor_tensor(out=q_rope, in0=q_rope, in1=tmpq, op=AL.add)

            k_rope = pa.tile([P, D], F16, tag="krope")
            tmpk = pa.tile([P, D], F16, tag="tmpk")
            nc.vector.scalar_tensor_tensor(
                out=tmpk[:, 0:H2], in0=k2[:, H2:D], scalar=-1.0,
                in1=sin_g[:, 0:H2], op0=AL.mult, op1=AL.mult,
            )
            nc.vector.tensor_tensor(
                out=tmpk[:, H2:D], in0=k2[:, 0:H2], in1=sin_g[:, H2:D], op=AL.mult
            )
            nc.vector.tensor_tensor(out=k_rope, in0=k2, in1=cos_g, op=AL.mult)
            nc.vector.tensor_tensor(out=k_rope, in0=k_rope, in1=tmpk, op=AL.add)

            nc.vector.tensor_copy(
                vv[:, g, :], qkv_sb[:, NH * D + D : NH * D + 2 * D]
            )
            return q_rope, k_rope

        def rope_transpose(g, q_rope, k_rope):
            """Transpose RoPE'd q/k into persistent qT_all / kT (PE, f16)."""
            tq_ps = ps_mega.tile([P, 4 * P], F16, tag="mega", name="tq")
            for h in range(NH):
                nc.tensor.transpose(
                    tq_ps[:, h * P : (h + 1) * P], q_rope[:, h, :], ident
                )
            nc.vector.tensor_copy(
                qT_all[:, :, g * P : (g + 1) * P],
                tq_ps.rearrange("p (h d) -> p h d", h=NH),
            )
            tk_ps = ps_mega.tile([P, 4 * P], F16, tag="mega", name="tk")
            nc.tensor.transpose(tk_ps[:, 0:P], k_rope, ident)
            nc.vector.tensor_copy(kT[:, g * P : (g + 1) * P], tk_ps[:, 0:P])

        # ================= phase B stages =================
        def scores_step(t, hp):
            """scoresT + exp for head-pair hp of macro tile t -> expst.

            expst[sk, ik, h2, q]: per key chunk ik, both heads of the pair.
            Diagonal chunks get a post-exp 0/1 triangle multiply (GpSimd);
            the odd diagonal chunk only computes the upper query half."""
            q0 = t * TQ
            nsk = 2 * (t + 1)
            expst = pb.tile([P, NSK, 2, TQ], F16, tag="expst", bufs=2)
            for ik in range(nsk):
                s_ps = ps_mega.tile([P, 2 * TQ], F32, tag="mega", name="s")
                s3 = s_ps.rearrange("p (h q) -> p h q", h=2)
                if ik == nsk - 1:  # odd diagonal: queries q0+128..q0+255 only
                    nc.tensor.matmul(
                        s3[:, :, P:TQ],
                        kT[:, ik * P : (ik + 1) * P],
                        qT_all[:, 2 * hp : 2 * hp + 2, q0 + P : q0 + TQ],
                        start=True, stop=True,
                    )
                    nc.scalar.activation(
                        out=expst[:, ik, :, P:TQ], in_=s3[:, :, P:TQ],
                        func=AF.Exp, scale=SCALE,
                    )
                    nc.vector.tensor_tensor(
                        out=expst[:, ik, :, P:TQ], in0=expst[:, ik, :, P:TQ],
                        in1=tri01, op=AL.mult,
                    )
                else:
                    nc.tensor.matmul(
                        s3,
                        kT[:, ik * P : (ik + 1) * P],
                        qT_all[:, 2 * hp : 2 * hp + 2, q0 : q0 + TQ],
                        start=True, stop=True,
                    )
                    nc.scalar.activation(
                        out=expst[:, ik, :, :], in_=s3, func=AF.Exp, scale=SCALE,
                    )
                    if ik == nsk - 2:  # even diagonal: lower-left triangle
                        nc.vector.tensor_tensor(
                            out=expst[:, ik, :, 0:P], in0=expst[:, ik, :, 0:P],
                            in1=tri01, op=AL.mult,
                        )
            return expst

        def dnpv_step(t, hp, expst, uT_t):
            """denominator + PV matmuls, then normalize into uT_t."""
            nsk = 2 * (t + 1)
            u_ps = ps_mega.tile([P, 2 * TQ], F32, tag="mega", name="u")
            den_ps = ps_mega.tile([P, 2 * TQ], F32, tag="mega", name="den")
            u3 = u_ps.rearrange("p (h q) -> p h q", h=2)
            d3 = den_ps.rearrange("p (h q) -> p h q", h=2)
            for ik in range(nsk):
                last = ik == nsk - 1
                rhs = expst[:, ik, :, P:TQ] if last else expst[:, ik, :, :]
                nc.tensor.matmul(
                    d3[:, :, P:TQ] if last else d3, ones, rhs,
                    start=(ik == 0), stop=last,
                )
            rec = pb.tile([P, 2 * TQ], F32, tag="rec", bufs=2)
            nc.vector.reciprocal_approx_fast(out=rec, in_=den_ps)
            if dbg:
                den_sb = pb.tile([P, 2 * TQ], F32, tag="densb", bufs=2)
                nc.scalar.activation(out=den_sb, in_=den_ps, func=AF.Copy)
                off = (t * 2 + hp) * 2 * TQ
                nc.gpsimd.dma_start(out=den_o[:, off : off + 2 * TQ], in_=den_sb)
                nc.gpsimd.dma_start(out=rec_o[:, off : off + 2 * TQ], in_=rec)
            for ik in range(nsk):
                last = ik == nsk - 1
                rhs = expst[:, ik, :, P:TQ] if last else expst[:, ik, :, :]
                nc.tensor.matmul(
                    u3[:, :, P:TQ] if last else u3,
                    vv[:, ik, :], rhs,
                    start=(ik == 0), stop=last,
                )
            nc.vector.tensor_tensor(
                out=uT_t[:, 2 * hp : 2 * hp + 2, :],
                in0=u3,
                in1=rec.rearrange("p (h q) -> p h q", h=2),
                op=AL.mult,
            )

        def wo_stage(t, uT_t):
            if dbg:
                off = t * NH * TQ
                nc.gpsimd.dma_start(
                    out=ut_o[:, off : off + NH * TQ],
                    in_=uT_t.rearrange("p h q -> p (h q)"),
                )
            for sub in range(2):
                g = 2 * t + sub
                y_sb = pb.tile([P, HID], F16, tag="ysb", bufs=2)
                for n in range(HID // 512):
                    y_ps = ps_mega.tile([P, 512], F32, tag="mega", name="y")
                    for h in range(NH):
                        nc.tensor.matmul(
                            y_ps,
                            uT_t[:, h, sub * P : (sub + 1) * P],
                            wo_sb[:, h, n * 512 : (n + 1) * 512],
                            start=(h == 0), stop=(h == NH - 1),
                        )
                    if n % 2 == 0:
                        nc.vector.tensor_copy(
                            y_sb[:, n * 512 : (n + 1) * 512], y_ps
                        )
                    else:
                        nc.scalar.activation(
                            out=y_sb[:, n * 512 : (n + 1) * 512], in_=y_ps,
                            func=AF.Copy,
                        )
                        nc.gpsimd.dma_start(
                            out=out_d[
                                g * P : (g + 1) * P, (n - 1) * 512 : (n + 1) * 512
                            ],
                            in_=y_sb[:, (n - 1) * 512 : (n + 1) * 512],
                        )

        # ================= driver =================
        ropes = [None] * NG
        pend = [None] * NG  # g -> [xTg, qkv_sb]

        def emit_phase_a(g):
            # PE-ready work (proj, ropeT) is emitted BEFORE the DMA-gated
            # transposes: the PE wait queue is only 4 deep, so a blocked
            # tp(g) would stall everything emitted after it.
            # wo DMAs go out at g=3/4: after most x chunks on the sync
            # queue (x arrival gates the fill phase) but before wo_stage(0)
            # is emitted at g=5.
            if g in (3, 4):
                emit_wo_dma()
                emit_wo_dma()
            if g >= 2:
                gg = g - 2
                sc = nc.named_scope(f"rope_{gg}"); sc.__enter__()
                ropes[gg] = rope_stage(gg, pend[gg][1])
                sc.__exit__(None, None, None)
            if g >= 1 and g - 1 < NG:
                gg = g - 1
                sc = nc.named_scope(f"proj_{gg}"); sc.__enter__()
                pend[gg][1] = proj(gg, pend[gg][0])
                sc.__exit__(None, None, None)
            if g >= 2:
                gg = g - 2
                sc = nc.named_scope(f"ropeT_{gg}"); sc.__enter__()
                rope_transpose(gg, *ropes[gg])
                sc.__exit__(None, None, None)
                pend[gg] = None
                ropes[gg] = None
            if g < NG:
                if g + 3 < NG:
                    emit_xdma(g + 3)
                sc = nc.named_scope(f"tp_{g}"); sc.__enter__()
                xTg = transposes(g)
                sc.__exit__(None, None, None)
                pend[g] = [xTg, None]

        steps = [(t, hp) for t in range(NT) for hp in range(2)]
        uts = {}
        att_i = [0]

        def emit_attention_step():
            # dnpv/wo (always PE-ready) go before the next scores step,
            # whose matmuls may still be blocked on ropeT of a later chunk.
            i = att_i[0]
            if i >= len(steps) + 1:
                return False
            if 1 <= i:
                t, hp = steps[i - 1]
                sc = nc.named_scope(f"dnpv_{t}_{hp}"); sc.__enter__()
                dnpv_step(t, hp, uts.pop((t, hp)), uts[t])
                sc.__exit__(None, None, None)
                if hp == 1:
                    sc = nc.named_scope(f"wo_{t}"); sc.__enter__()
                    wo_stage(t, uts.pop(t))
                    sc.__exit__(None, None, None)
            if i < len(steps):
                t, hp = steps[i]
                if hp == 0:
                    uts[t] = pb.tile([P, NH, TQ], F16, tag="uT", name=f"uT{t}")
                sc = nc.named_scope(f"sc_{t}_{hp}"); sc.__enter__()
                uts[(t, hp)] = scores_step(t, hp)
                sc.__exit__(None, None, None)
            att_i[0] += 1
            return True

        for g in range(NG + 2):
            emit_phase_a(g)
            done_g = g - 2  # ropeT for this chunk just emitted
            while att_i[0] < len(steps) + 1:
                i = att_i[0]
                if i < len(steps):
                    t, _hp = steps[i]
                    if 2 * t + 1 > done_g:
                        break
                emit_attention_step()
        while emit_attention_step():
            pass

    nc.compile()
    return nc


def shard_inputs(x, cos, sin, wq, wk, wv, wo):
    """Build per-core input maps: core = b*4 + g. All f16."""
    f16 = np.float16
    in_maps = []
    for c in range(N_CORES):
        b, g = divmod(c, N_KV)
        in_maps.append(
            {
                "x": np.ascontiguousarray(x[b], dtype=f16),
                "cos": np.ascontiguousarray(cos, dtype=f16),
                "sin": np.ascontiguousarray(sin, dtype=f16),
                "wq": np.ascontiguousarray(
                    wq[:, g * NH * D : (g + 1) * NH * D], dtype=f16
                ),
                "wk": np.ascontiguousarray(wk[:, g * D : (g + 1) * D], dtype=f16),
                "wv": np.ascontiguousarray(wv[:, g * D : (g + 1) * D], dtype=f16),
                "wo": np.ascontiguousarray(
                    wo[g * NH * D : (g + 1) * NH * D, :], dtype=f16
                ),
            }
        )
    return in_maps


_NC_CACHE = {}


def get_nc():
    if "nc" not in _NC_CACHE:
        _NC_CACHE["nc"] = build_nc()
    return _NC_CACHE["nc"]


def kernel(x, cos, sin, wq, wk, wv, wo, _trace=False):
    from concourse.bass_utils import run_bass_kernel_spmd

    x = np.asarray(x, dtype=np.float32)
    cos = np.asarray(cos, dtype=np.float32)
    sin = np.asarray(sin, dtype=np.float32)
    wq = np.asarray(wq, dtype=np.float32)
    wk = np.asarray(wk, dtype=np.float32)
    wv = np.asarray(wv, dtype=np.float32)
    wo = np.asarray(wo, dtype=np.float32)

    nc = get_nc()
    in_maps = shard_inputs(x, cos, sin, wq, wk, wv, wo)
    res = run_bass_kernel_spmd(nc, in_maps, list(range(N_CORES)), trace=_trace)
    parts = [np.asarray(res.results[c]["out"], dtype=np.float32) for c in range(N_CORES)]
    y = np.stack(
        [sum(parts[b * N_KV + g] for g in range(N_KV)) for b in range(B)], axis=0
    )
    if _trace:
        kernel.last_result = res
    return y



# revision 4
# speedup vs baseline: 1.0783x; 1.0783x over previous
"""Trainium2 Bass kernel for GQA attention with RoPE (B=2, S=1024, HID=2048,
16 q heads / 4 kv heads, head dim 128, causal).

Sharding: 8 cores = 2 batches x 4 kv-head groups. Core c = b*4 + g handles
batch b and kv head g (query heads 4g..4g+3). Each core computes a partial
output y_part = attn_heads @ wo_shard; the host sums the 4 partials per batch.

v4: host pre-transposes x into a per-chunk-contiguous [P, NG, KC, P] layout,
removing all on-device x transposes (PE) and their PSUM copy-outs (DVE/ACT).
Warmup block interleaves f16 and fp8-DoubleRow dummy matmuls to measure the
HW DoubleRow rate from the trace.
"""

import sys

import numpy as np

for _p in ("/opt/trn_rl_repo", "/root/.axon_site/_ro/trn_rl_repo"):
    if _p not in sys.path:
        sys.path.append(_p)

from contextlib import ExitStack

import concourse.bass as bass
import concourse.mybir as mybir
from concourse import bacc
from concourse.masks import make_identity
from concourse.tile import TileContext

P = 128           # partitions / head dim / seq chunk
S = 1024          # sequence length
HID = 2048        # model dim
NH = 4            # query heads per core
D = 128           # head dim
TQ = 256          # query macro-tile
NT = S // TQ      # 4 macro tiles
KC = HID // P     # 16 contraction chunks
NSK = S // P      # 8 key chunks
NG = S // P       # 8 row chunks
F16 = mybir.dt.float16
F32 = mybir.dt.float32
F8 = mybir.dt.float8e4
SCALE = 1.0 / float(np.sqrt(D))
AL = mybir.AluOpType
AF = mybir.ActivationFunctionType
PM = mybir.MatmulPerfMode

N_CORES = 8
B = 2
N_KV = 4
H2 = D // 2


def build_nc(dbg=False):
    nc = bacc.Bacc("TRN2", target_bir_lowering=False, debug=False)
    # x pre-transposed+chunked by host: xt[p, g, c, sq] = x[g*P+sq, c*P+p]
    xt_d = nc.declare_dram_parameter("xt", [P, NG * KC * P], F16, isOutput=False)
    cos_d = nc.declare_dram_parameter("cos", [S, D], F16, isOutput=False)
    sin_d = nc.declare_dram_parameter("sin", [S, D], F16, isOutput=False)
    wq_d = nc.declare_dram_parameter("wq", [HID, NH * D], F16, isOutput=False)
    wk_d = nc.declare_dram_parameter("wk", [HID, D], F16, isOutput=False)
    wv_d = nc.declare_dram_parameter("wv", [HID, D], F16, isOutput=False)
    wo_d = nc.declare_dram_parameter("wo", [NH * D, HID], F16, isOutput=False)
    out_d = nc.declare_dram_parameter("out", [S, HID], F16, isOutput=True)

    with TileContext(nc) as tc, ExitStack() as ctx:
        consts = ctx.enter_context(tc.tile_pool(name="consts", bufs=1))
        wpool = ctx.enter_context(tc.tile_pool(name="wpool", bufs=1))
        persist = ctx.enter_context(tc.tile_pool(name="persist", bufs=1))

        # ---- constants ----
        ident_f32 = consts.tile([P, P], F32, tag="ident_f32")
        make_identity(nc, ident_f32)
        ident = consts.tile([P, P], F16, tag="ident")
        nc.vector.tensor_copy(ident, ident_f32)
        ones = consts.tile([P, P], F16, tag="ones")
        nc.vector.memset(ones, 1.0)
        # fp8 dummies for the DoubleRow timing probe
        dr_a = consts.tile([P, 2, P], F8, tag="dr_a")
        nc.vector.memset(dr_a, 1.0)
        dr_b = consts.tile([P, 2, 512], F8, tag="dr_b")
        nc.vector.memset(dr_b, 1.0)
        warm16 = consts.tile([P, 512], F16, tag="warm16")
        nc.vector.memset(warm16, 1.0)
        # causal 0/1 triangle: tri01[k, h, q] = (q >= k), f16, shared by both
        # diagonal chunks of every macro tile
        tri01 = consts.tile([P, 2, P], F16, tag="tri01")
        nc.gpsimd.memset(tri01, 1.0)
        nc.gpsimd.affine_select(
            out=tri01, in_=tri01, compare_op=AL.is_ge, fill=0.0,
            base=0, pattern=[[0, 2], [1, P]], channel_multiplier=-1,
        )

        # ---- persistent weights / tables / activations ----
        wq_sb = wpool.tile([P, KC, NH * D], F16, tag="wq")
        wq_r = wq_d[:].rearrange("(c p) n -> p c n", p=P)
        wkv_sb = wpool.tile([P, KC, 2 * D], F16, tag="wkv")
        wo_sb = wpool.tile([P, NH, HID], F16, tag="wo")
        wo_r = wo_d[:].rearrange("(h p) n -> p h n", p=P)
        cos_sb = wpool.tile([P, NG, D], F16, tag="cos")
        sin_sb = wpool.tile([P, NG, D], F16, tag="sin")

        qT_all = persist.tile([P, NH, S], F16, tag="qT")    # [d, h, sq]
        kT = persist.tile([P, S], F16, tag="kT")            # [d, sk]
        vv = persist.tile([P, NSK, D], F16, tag="vv")       # v natural [sk, d]

        # ---- pools ----
        pa = ctx.enter_context(tc.tile_pool(name="pa", bufs=2))
        pb = ctx.enter_context(tc.tile_pool(name="pb", bufs=2))
        ps_mega = ctx.enter_context(tc.tile_pool(name="ps_mega", bufs=7, space="PSUM"))
        ps_qkv = ctx.enter_context(tc.tile_pool(name="ps_qkv", bufs=1, space="PSUM"))

        # warm the PE clock gate while initial DMAs land; interleave f16 and
        # fp8-DoubleRow dummies so the trace reveals the HW DoubleRow rate.
        warm_ps = ps_mega.tile([P, 512], F32, tag="mega", name="warm")
        for _ in range(8):
            nc.tensor.matmul(warm_ps[:, 0:P], ones, ones, start=True, stop=True)
        for _ in range(6):
            nc.tensor.matmul(warm_ps, ones, warm16, start=True, stop=True)
            nc.tensor.matmul(
                warm_ps, dr_a, dr_b, start=True, stop=True, perf_mode=PM.DoubleRow
            )
        warm_drain = pa.tile([P, 4], F32, tag="warmdrain", bufs=1)
        nc.vector.tensor_copy(warm_drain, warm_ps[:, 0:4])

        # ---- DMAs: xt per-chunk (contiguous 4KB/partition, sync queue),
        # weights on the sync queue, cos/sin on the scalar queue. ----
        x_tiles = [None] * NG

        def emit_xdma(g):
            xTg = pa.tile([P, KC, P], F16, tag="xT", bufs=4)
            nc.sync.dma_start(
                out=xTg.rearrange("p c d -> p (c d)"),
                in_=xt_d[:, g * KC * P : (g + 1) * KC * P],
            )
            x_tiles[g] = xTg

        emit_xdma(0)
        nc.scalar.dma_start(
            out=cos_sb, in_=cos_d[:].rearrange("(c p) d -> p c d", p=P)
        )
        nc.scalar.dma_start(
            out=sin_sb, in_=sin_d[:].rearrange("(c p) d -> p c d", p=P)
        )
        emit_xdma(1)
        nc.sync.dma_start(out=wq_sb[:, 0:8, :], in_=wq_r[:, 0:8, :])
        nc.sync.dma_start(out=wq_sb[:, 8:16, :], in_=wq_r[:, 8:16, :])
        emit_xdma(2)
        nc.sync.dma_start(
            out=wkv_sb[:, :, 0:D], in_=wk_d[:].rearrange("(c p) n -> p c n", p=P)
        )
        nc.sync.dma_start(
            out=wkv_sb[:, :, D : 2 * D],
            in_=wv_d[:].rearrange("(c p) n -> p c n", p=P),
        )
        emit_xdma(3)
        wo_next = [0]

        def emit_wo_dma():
            h = wo_next[0]
            if h < NH:
                nc.sync.dma_start(out=wo_sb[:, h, :], in_=wo_r[:, h, :])
                wo_next[0] += 1

        def bcast_h(ap2d, n):
            """[P, w] slice -> [P, n, w] broadcast AP (0-stride head dim)."""
            return ap2d.rearrange("p (o w) -> p o w", o=1).to_broadcast(
                [P, n, ap2d.shape[-1]]
            )

        # ================= phase A stages =================
        def proj(g):
            """q and kv projections for chunk g (PE, accumulating).
            q uses the dedicated 1-bank pool; kv borrows a mega slot so the
            attention phase gets a 7-deep mega rotation."""
            xTg = x_tiles[g]
            q_ps = ps_qkv.tile([P, NH * D], F32, tag="qkv")
            kv_ps = ps_mega.tile([P, 512], F32, tag="mega", name="kv")[:, 0 : 2 * D]
            for c in range(KC):
                nc.tensor.matmul(
                    q_ps, xTg[:, c, :], wq_sb[:, c, :],
                    start=(c == 0), stop=(c == KC - 1),
                )
            for c in range(KC):
                nc.tensor.matmul(
                    kv_ps, xTg[:, c, :], wkv_sb[:, c, :],
                    start=(c == 0), stop=(c == KC - 1),
                )
            qkv_sb = pa.tile([P, NH * D + 2 * D], F16, tag="qkvsb")
            nc.scalar.activation(out=qkv_sb[:, 0 : NH * D], in_=q_ps, func=AF.Copy)
            nc.scalar.activation(
                out=qkv_sb[:, NH * D : NH * D + 2 * D], in_=kv_ps, func=AF.Copy
            )
            return qkv_sb

        def rope_stage(g, qkv_sb):
            """RoPE on q heads (one 4-head strided pass) + k; v copy-out."""
            q3 = qkv_sb[:, 0 : NH * D].rearrange("p (h d) -> p h d", h=NH)
            k2 = qkv_sb[:, NH * D : NH * D + D]
            cos_g = cos_sb[:, g, :]
            sin_g = sin_sb[:, g, :]

            q_rope = pa.tile([P, NH, D], F16, tag="qrope")
            tmpq = pa.tile([P, NH, D], F16, tag="tmpq")
            nc.vector.scalar_tensor_tensor(
                out=tmpq[:, :, 0:H2], in0=q3[:, :, H2:D], scalar=-1.0,
                in1=bcast_h(sin_g[:, 0:H2], NH), op0=AL.mult, op1=AL.mult,
            )
            nc.vector.tensor_tensor(
                out=tmpq[:, :, H2:D], in0=q3[:, :, 0:H2],
                in1=bcast_h(sin_g[:, H2:D], NH), op=AL.mult,
            )
            nc.vector.tensor_tensor(
                out=q_rope, in0=q3, in1=bcast_h(cos_g, NH), op=AL.mult
            )
            nc.vector.tensor_tensor(out=q_rope, in0=q_rope, in1=tmpq, op=AL.add)

            k_rope = pa.tile([P, D], F16, tag="krope")
            tmpk = pa.tile([P, D], F16, tag="tmpk")
            nc.vector.scalar_tensor_tensor(
                out=tmpk[:, 0:H2], in0=k2[:, H2:D], scalar=-1.0,
                in1=sin_g[:, 0:H2], op0=AL.mult, op1=AL.mult,
            )
            nc.vector.tensor_tensor(
                out=tmpk[:, H2:D], in0=k2[:, 0:H2], in1=sin_g[:, H2:D], op=AL.mult
            )
            nc.vector.tensor_tensor(out=k_rope, in0=k2, in1=cos_g, op=AL.mult)
            nc.vector.tensor_tensor(out=k_rope, in0=k_rope, in1=tmpk, op=AL.add)

            nc.vector.tensor_copy(
                vv[:, g, :], qkv_sb[:, NH * D + D : NH * D + 2 * D]
            )
            return q_rope, k_rope

        def rope_transpose(g, q_rope, k_rope):
            """Transpose RoPE'd q/k into persistent qT_all / kT (PE, f16)."""
            tq_ps = ps_mega.tile([P, 4 * P], F16, tag="mega", name="tq")
            for h in range(NH):
                nc.tensor.transpose(
                    tq_ps[:, h * P : (h + 1) * P], q_rope[:, h, :], ident
                )
            nc.vector.tensor_copy(
                qT_all[:, :, g * P : (g + 1) * P],
                tq_ps.rearrange("p (h d) -> p h d", h=NH),
            )
            tk_ps = ps_mega.tile([P, 4 * P], F16, tag="mega", name="tk")
            nc.tensor.transpose(tk_ps[:, 0:P], k_rope, ident)
            nc.vector.tensor_copy(kT[:, g * P : (g + 1) * P], tk_ps[:, 0:P])

        # ================= phase B stages =================
        def scores_step(t, hp):
            """scoresT + exp for head-pair hp of macro tile t -> expst.

            expst[sk, ik, h2, q]: per key chunk ik, both heads of the pair.
            Diagonal chunks get a post-exp 0/1 triangle multiply (GpSimd);
            the odd diagonal chunk only computes the upper query half."""
            q0 = t * TQ
            nsk = 2 * (t + 1)
            expst = pb.tile([P, NSK, 2, TQ], F16, tag="expst", bufs=2)
            for ik in range(nsk):
                s_ps = ps_mega.tile([P, 2 * TQ], F32, tag="mega", name="s")
                s3 = s_ps.rearrange("p (h q) -> p h q", h=2)
                if ik == nsk - 1:  # odd diagonal: queries q0+128..q0+255 only
                    nc.tensor.matmul(
                        s3[:, :, P:TQ],
                        kT[:, ik * P : (ik + 1) * P],
                        qT_all[:, 2 * hp : 2 * hp + 2, q0 + P : q0 + TQ],
                        start=True, stop=True,
                    )
                    nc.scalar.activation(
                        out=expst[:, ik, :, P:TQ], in_=s3[:, :, P:TQ],
                        func=AF.Exp, scale=SCALE,
                    )
                    nc.vector.tensor_tensor(
                        out=expst[:, ik, :, P:TQ], in0=expst[:, ik, :, P:TQ],
                        in1=tri01, op=AL.mult,
                    )
                else:
                    nc.tensor.matmul(
                        s3,
                        kT[:, ik * P : (ik + 1) * P],
                        qT_all[:, 2 * hp : 2 * hp + 2, q0 : q0 + TQ],
                        start=True, stop=True,
                    )
                    nc.scalar.activation(
                        out=expst[:, ik, :, :], in_=s3, func=AF.Exp, scale=SCALE,
                    )
                    if ik == nsk - 2:  # even diagonal: lower-left triangle
                        nc.vector.tensor_tensor(
                            out=expst[:, ik, :, 0:P], in0=expst[:, ik, :, 0:P],
                            in1=tri01, op=AL.mult,
                        )
            return expst

        def dnpv_step(t, hp, expst, uT_t):
            """denominator + PV matmuls, then normalize into uT_t."""
            nsk = 2 * (t + 1)
            u_ps = ps_mega.tile([P, 2 * TQ], F32, tag="mega", name="u")
            den_ps = ps_mega.tile([P, 2 * TQ], F32, tag="mega", name="den")
            u3 = u_ps.rearrange("p (h q) -> p h q", h=2)
            d3 = den_ps.rearrange("p (h q) -> p h q", h=2)
            for ik in range(nsk):
                last = ik == nsk - 1
                rhs = expst[:, ik, :, P:TQ] if last else expst[:, ik, :, :]
                nc.tensor.matmul(
                    d3[:, :, P:TQ] if last else d3, ones, rhs,
                    start=(ik == 0), stop=last,
                )
            rec = pb.tile([P, 2 * TQ], F32, tag="rec", bufs=2)
            nc.vector.reciprocal_approx_fast(out=rec, in_=den_ps)
            for ik in range(nsk):
                last = ik == nsk - 1
                rhs = expst[:, ik, :, P:TQ] if last else expst[:, ik, :, :]
                nc.tensor.matmul(
                    u3[:, :, P:TQ] if last else u3,
                    vv[:, ik, :], rhs,
                    start=(ik == 0), stop=last,
                )
            nc.vector.tensor_tensor(
                out=uT_t[:, 2 * hp : 2 * hp + 2, :],
                in0=u3,
                in1=rec.rearrange("p (h q) -> p h q", h=2),
                op=AL.mult,
            )

        def wo_stage(t, uT_t):
            for sub in range(2):
                g = 2 * t + sub
                y_sb = pb.tile([P, HID], F16, tag="ysb", bufs=2)
                for n in range(HID // 512):
                    y_ps = ps_mega.tile([P, 512], F32, tag="mega", name="y")
                    for h in range(NH):
                        nc.tensor.matmul(
                            y_ps,
                            uT_t[:, h, sub * P : (sub + 1) * P],
                            wo_sb[:, h, n * 512 : (n + 1) * 512],
                            start=(h == 0), stop=(h == NH - 1),
                        )
                    if n % 2 == 0:
                        nc.vector.tensor_copy(
                            y_sb[:, n * 512 : (n + 1) * 512], y_ps
                        )
                    else:
                        nc.scalar.activation(
                            out=y_sb[:, n * 512 : (n + 1) * 512], in_=y_ps,
                            func=AF.Copy,
                        )
                        nc.gpsimd.dma_start(
                            out=out_d[
                                g * P : (g + 1) * P, (n - 1) * 512 : (n + 1) * 512
                            ],
                            in_=y_sb[:, (n - 1) * 512 : (n + 1) * 512],
                        )

        # ================= driver =================
        ropes = [None] * NG
        qkvs = [None] * NG

        def emit_phase_a(g):
            # PE-ready work (ropeT) is emitted BEFORE the DMA-gated proj of
            # the current chunk: the PE wait queue is only 4 deep, so a
            # blocked proj(g) would stall everything emitted after it.
            # wo DMAs go out at g=3/4: after most x chunks on the sync
            # queue (x arrival gates the fill phase) but before wo_stage(0)
            # is emitted at g=5.
            if g in (3, 4):
                emit_wo_dma()
                emit_wo_dma()
            if g >= 1:
                gg = g - 1
                sc = nc.named_scope(f"rope_{gg}"); sc.__enter__()
                ropes[gg] = rope_stage(gg, qkvs[gg])
                sc.__exit__(None, None, None)
                sc = nc.named_scope(f"ropeT_{gg}"); sc.__enter__()
                rope_transpose(gg, *ropes[gg])
                sc.__exit__(None, None, None)
                ropes[gg] = None
                qkvs[gg] = None
            if g < NG:
                if g + 4 < NG:
                    emit_xdma(g + 4)
                sc = nc.named_scope(f"proj_{g}"); sc.__enter__()
                qkvs[g] = proj(g)
                sc.__exit__(None, None, None)

        steps = [(t, hp) for t in range(NT) for hp in range(2)]
        uts = {}
        att_i = [0]

        def emit_attention_step():
            # dnpv/wo (always PE-ready) go before the next scores step,
            # whose matmuls may still be blocked on ropeT of a later chunk.
            i = att_i[0]
            if i >= len(steps) + 1:
                return False
            if 1 <= i:
                t, hp = steps[i - 1]
                sc = nc.named_scope(f"dnpv_{t}_{hp}"); sc.__enter__()
                dnpv_step(t, hp, uts.pop((t, hp)), uts[t])
                sc.__exit__(None, None, None)
                if hp == 1:
                    sc = nc.named_scope(f"wo_{t}"); sc.__enter__()
                    wo_stage(t, uts.pop(t))
                    sc.__exit__(None, None, None)
            if i < len(steps):
                t, hp = steps[i]
                if hp == 0:
                    uts[t] = pb.tile([P, NH, TQ], F16, tag="uT", name=f"uT{t}")
                sc = nc.named_scope(f"sc_{t}_{hp}"); sc.__enter__()
                uts[(t, hp)] = scores_step(t, hp)
                sc.__exit__(None, None, None)
            att_i[0] += 1
            return True

        for g in range(NG + 1):
            emit_phase_a(g)
            done_g = g - 1  # ropeT for this chunk just emitted
            while att_i[0] < len(steps) + 1:
                i = att_i[0]
                if i < len(steps):
                    t, _hp = steps[i]
                    if 2 * t + 1 > done_g:
                        break
                emit_attention_step()
        while emit_attention_step():
            pass

    nc.compile()
    return nc


def shard_inputs(x, cos, sin, wq, wk, wv, wo):
    """Build per-core input maps: core = b*4 + g. All f16.
    x is pre-transposed+chunked: xt[p, g*KC*P + c*P + sq] = x[g*P+sq, c*P+p]."""
    f16 = np.float16
    xts = []
    for b in range(B):
        xb = np.asarray(x[b], dtype=f16).reshape(NG, P, KC, P)
        xts.append(np.ascontiguousarray(xb.transpose(3, 0, 2, 1)).reshape(P, NG * KC * P))
    in_maps = []
    for c in range(N_CORES):
        b, g = divmod(c, N_KV)
        in_maps.append(
            {
                "xt": xts[b],
                "cos": np.ascontiguousarray(cos, dtype=f16),
                "sin": np.ascontiguousarray(sin, dtype=f16),
                "wq": np.ascontiguousarray(
                    wq[:, g * NH * D : (g + 1) * NH * D], dtype=f16
                ),
                "wk": np.ascontiguousarray(wk[:, g * D : (g + 1) * D], dtype=f16),
                "wv": np.ascontiguousarray(wv[:, g * D : (g + 1) * D], dtype=f16),
                "wo": np.ascontiguousarray(
                    wo[g * NH * D : (g + 1) * NH * D, :], dtype=f16
                ),
            }
        )
    return in_maps


_NC_CACHE = {}


def get_nc():
    if "nc" not in _NC_CACHE:
        _NC_CACHE["nc"] = build_nc()
    return _NC_CACHE["nc"]


def kernel(x, cos, sin, wq, wk, wv, wo, _trace=False):
    from concourse.bass_utils import run_bass_kernel_spmd

    x = np.asarray(x, dtype=np.float32)
    cos = np.asarray(cos, dtype=np.float32)
    sin = np.asarray(sin, dtype=np.float32)
    wq = np.asarray(wq, dtype=np.float32)
    wk = np.asarray(wk, dtype=np.float32)
    wv = np.asarray(wv, dtype=np.float32)
    wo = np.asarray(wo, dtype=np.float32)

    nc = get_nc()
    in_maps = shard_inputs(x, cos, sin, wq, wk, wv, wo)
    res = run_bass_kernel_spmd(nc, in_maps, list(range(N_CORES)), trace=_trace)
    parts = [np.asarray(res.results[c]["out"], dtype=np.float32) for c in range(N_CORES)]
    y = np.stack(
        [sum(parts[b * N_KV + g] for g in range(N_KV)) for b in range(B)], axis=0
    )
    if _trace:
        kernel.last_result = res
    return y
